# revision 1
# baseline (speedup 1.0000x reference)
"""Shared builder for the HGraphConv Bass kernel.

Design (per core, dst-sharded — no collectives needed):
  Each relation (src table X [N_src,128], edge list (src,dst), dst space N_dst)
  is processed as:  out[d] = leaky( (sum_e v_e * X[src_e]) @ W + b ) @ Wl + bl
  where v_e = rsqrt(deg_src[src_e]) * rsqrt(deg_dst[dst_e]).

  dst space is split evenly over 8 cores; each core's slice is cut into
  128-row blocks. Host buckets edges by (core, block), pads each bucket to a
  multiple of 128 edges (padding edges have v=0), and transposes into
  [128, C] panels so edge k = c*128+p lives at partition p, column c.

  Device per block:
    idx/colv panel slices DMA'd just-in-time (keeps per-instruction sem waits low)
    msg  [128,C,128] bf16 <- indirect DMA gather rows X[src] (cast f32->bf16)
    per chunk c:
      oh [128,128] bf16 = (iota == col[:,c]) * v[:,c]      (one DVE op)
      psum_aggT [128f,128d] += msg[:,c,:].T @ oh           (PE matmul)
    aggT bf16 <- psum (ACT copy)
    h1T [128,128] = W.T @ aggT  (PE)
    z = h1T+b, rz = relu(h1T+b) (ACT, bf16)
    outT [64,128] = z @ (0.01*Wl) + rz @ (0.99*Wl) (PE, = leaky@Wl), + bl (DVE)
    DMA out transposed; host transposes back and assembles.
"""

import math
import sys

sys.path.insert(0, "/opt/trn_rl_repo")
sys.path.insert(0, "/root/.axon_site/_ro/trn_rl_repo")

import numpy as np

import concourse.bass as bass
import concourse.tile as tile
from concourse import bacc
from concourse import mybir
from concourse.bass import IndirectOffsetOnAxis

P = 128
N_CORES = 8


def pack_relation(src, dst, n_src, n_dst, n_cores=N_CORES):
    """Bucket edges by (core, dst-block); returns per-core panels."""
    assert n_dst % n_cores == 0
    per_core = n_dst // n_cores
    nblk = math.ceil(per_core / P)

    deg_s = np.maximum(np.bincount(src, minlength=n_src), 1).astype(np.float64)
    deg_d = np.maximum(np.bincount(dst, minlength=n_dst), 1).astype(np.float64)
    v_all = (1.0 / np.sqrt(deg_s[src] * deg_d[dst])).astype(np.float32)

    core = dst // per_core
    rem = dst - core * per_core
    b_loc = rem // P
    col = (rem % P).astype(np.float32)

    group = core * nblk + b_loc  # [E]
    gcounts = np.bincount(group, minlength=n_cores * nblk).reshape(n_cores, nblk)
    C_b = np.maximum(np.ceil(gcounts / P).max(axis=0).astype(np.int64), 1)
    totc = int(C_b.sum())
    offs = np.concatenate([[0], np.cumsum(C_b)])[:-1]

    order = np.argsort(group, kind="stable")
    g_sorted = group[order]
    starts = np.concatenate([[0], np.cumsum(gcounts.ravel())])[:-1]
    pos = np.arange(len(src)) - starts[g_sorted]

    e_core = g_sorted // nblk
    e_blk = g_sorted % nblk
    e_chunk = pos // P
    e_p = pos % P
    e_col_idx = offs[e_blk] + e_chunk

    idx_arr = np.zeros((n_cores, P, totc), np.int32)
    col_arr = np.zeros((n_cores, P, totc), np.float32)
    v_arr = np.zeros((n_cores, P, totc), np.float32)
    idx_arr[e_core, e_p, e_col_idx] = src[order]
    col_arr[e_core, e_p, e_col_idx] = col[order]
    v_arr[e_core, e_p, e_col_idx] = v_all[order]

    return dict(
        counts=C_b.astype(int).tolist(),
        idx=idx_arr,
        col=col_arr,
        v=v_arr,
        nblk=nblk,
        totc=totc,
        per_core=per_core,
    )


def build_program(rels):
    """rels: list of dicts with keys: name, tab_shape, counts, totc, nblk.

    Inputs per relation r: tab_{nm} f32 [rows,128], idx_{nm} i32 [128,totc],
    colv_{nm} f32 [128,totc,2], wb_{nm} bf16 [128,256] (W|0.01Wl|0.99Wl),
    fb_{nm} f32 [128,2] (b | bl-padded).  Shared: iota bf16 [128,128].
    Output per relation: outT_{nm} f32 [64, nblk*128].
    """
    fp32 = mybir.dt.float32
    bf16 = mybir.dt.bfloat16
    i32 = mybir.dt.int32

    nc = bacc.Bacc(None)

    iota_d = nc.dram_tensor("iota", [P, P], bf16, kind="ExternalInput")
    tens = {}
    for r in rels:
        nm = r["name"]
        totc = r["totc"]
        tens[nm] = dict(
            tab=nc.dram_tensor(f"tab_{nm}", list(r["tab_shape"]), fp32, kind="ExternalInput"),
            idx=nc.dram_tensor(f"idx_{nm}", [P, totc], i32, kind="ExternalInput"),
            colv=nc.dram_tensor(f"colv_{nm}", [P, totc, 2], fp32, kind="ExternalInput"),
            wb=nc.dram_tensor(f"wb_{nm}", [P, 256], bf16, kind="ExternalInput"),
            fb=nc.dram_tensor(f"fb_{nm}", [P, 2], fp32, kind="ExternalInput"),
            outT=nc.dram_tensor(f"outT_{nm}", [64, r["nblk"] * P], fp32, kind="ExternalOutput"),
        )

    with tile.TileContext(nc) as tc:
        with (
            tc.tile_pool(name="res", bufs=1) as res,
            tc.tile_pool(name="msg", bufs=3) as msgp,
            tc.tile_pool(name="oh", bufs=6) as ohp,
            tc.tile_pool(name="mid", bufs=3) as midp,
            tc.tile_pool(name="obuf", bufs=3) as obufp,
            tc.tile_pool(name="ps_agg", bufs=2, space="PSUM") as ps_agg,
            tc.tile_pool(name="ps_mm", bufs=2, space="PSUM") as ps_mm,
        ):
            iota_t0 = res.tile([P, P], bf16, name="iota_t0")
            nc.sync.dma_start(out=iota_t0[:], in_=iota_d[:])
            iota_t = res.tile([P, P], bf16, name="iota_t")
            nc.vector.tensor_copy(iota_t[:], iota_t0[:])

            rt = {}
            for r in rels:
                nm = r["name"]
                d = tens[nm]
                t = dict(
                    wb=res.tile([P, 256], bf16, tag=f"wb_{nm}", name=f"wbt_{nm}"),
                    fb=res.tile([P, 2], fp32, tag=f"fb_{nm}", name=f"fbt0_{nm}"),
                )
                for k in t:
                    nc.sync.dma_start(out=t[k][:], in_=d[k][:])
                fb_a = res.tile([P, 2], fp32, tag=f"fba_{nm}", name=f"fba_{nm}")
                nc.scalar.copy(fb_a[:], t["fb"][:])
                t["fb"] = fb_a
                rt[nm] = t

            for r in rels:
                nm = r["name"]
                d = tens[nm]
                t = rt[nm]
                counts = r["counts"]
                off = 0
                for bi, C in enumerate(counts):
                    idx_t = msgp.tile([P, C], i32, tag="idx", name="idx_t")
                    nc.sync.dma_start(out=idx_t[:], in_=d["idx"][:, off : off + C])
                    colv_t0 = msgp.tile([P, C, 2], fp32, tag="colv0", name="colv_t0")
                    nc.sync.dma_start(out=colv_t0[:], in_=d["colv"][:, off : off + C, :])
                    colv_t = msgp.tile([P, C, 2], fp32, tag="colv", name="colv_t")
                    nc.vector.tensor_copy(colv_t[:], colv_t0[:])
                    msg = msgp.tile([P, C, P], bf16, tag="msg")
                    for c in range(C):
                        nc.gpsimd.indirect_dma_start(
                            out=msg[:, c, :],
                            out_offset=None,
                            in_=d["tab"][:],
                            in_offset=IndirectOffsetOnAxis(ap=idx_t[:, c : c + 1], axis=0),
                        )
                    agg = ps_agg.tile([P, P], mybir.dt.float32, tag="agg")
                    for c in range(C):
                        oh = ohp.tile([P, P], bf16, tag="oh")
                        nc.vector.tensor_scalar(
                            oh[:],
                            iota_t[:],
                            colv_t[:, c, 0:1],
                            colv_t[:, c, 1:2],
                            mybir.AluOpType.is_equal,
                            mybir.AluOpType.mult,
                        )
                        nc.tensor.matmul(
                            out=agg[:],
                            lhsT=msg[:, c, :],
                            rhs=oh[:],
                            start=(c == 0),
                            stop=(c == C - 1),
                        )
                    aggT = midp.tile([P, P], bf16, tag="aggT")
                    nc.scalar.copy(aggT[:], agg[:])
                    h1ps = ps_mm.tile([P, P], mybir.dt.float32, tag="h1ps")
                    nc.tensor.matmul(out=h1ps[:], lhsT=t["wb"][:, 0:P], rhs=aggT[:], start=True, stop=True)
                    z = midp.tile([P, P], bf16, tag="z")
                    nc.scalar.activation(
                        z[:], h1ps[:], mybir.ActivationFunctionType.Identity,
                        bias=t["fb"][:, 0:1], scale=1.0,
                    )
                    rz = midp.tile([P, P], bf16, tag="rz")
                    nc.scalar.activation(
                        rz[:], h1ps[:], mybir.ActivationFunctionType.Relu,
                        bias=t["fb"][:, 0:1], scale=1.0,
                    )
                    ops_ = ps_mm.tile([64, P], mybir.dt.float32, tag="ops")
                    nc.tensor.matmul(out=ops_[:], lhsT=t["wb"][:, P : P + 64], rhs=z[:], start=True, stop=False)
                    nc.tensor.matmul(out=ops_[:], lhsT=t["wb"][:, P + 64 : P + 128], rhs=rz[:], start=False, stop=True)
                    ob = obufp.tile([64, P], mybir.dt.float32, tag="ob")
                    nc.scalar.activation(
                        ob[:], ops_[:], mybir.ActivationFunctionType.Identity,
                        bias=t["fb"][:64, 1:2], scale=1.0,
                    )
                    nc.sync.dma_start(
                        out=d["outT"][:, bi * P : (bi + 1) * P], in_=ob[:]
                    )
                    off += C
    nc.compile()
    return nc


def make_rel_inputs(nm, pk, x, W, b, Wl, bl, core):
    """Build the per-core input dict entries for one relation."""
    import ml_dtypes

    wb = np.zeros((P, 256), ml_dtypes.bfloat16)
    wb[:, 0:P] = W.astype(ml_dtypes.bfloat16)
    wb[:, P : P + 64] = (0.01 * Wl).astype(ml_dtypes.bfloat16)
    wb[:, P + 64 : P + 128] = (0.99 * Wl).astype(ml_dtypes.bfloat16)
    fb = np.zeros((P, 2), np.float32)
    fb[:, 0] = b
    fb[:64, 1] = bl
    colv = np.stack([pk["col"][core], pk["v"][core]], axis=-1).astype(np.float32)
    return {
        f"tab_{nm}": x,
        f"idx_{nm}": pk["idx"][core],
        f"colv_{nm}": np.ascontiguousarray(colv),
        f"wb_{nm}": wb,
        f"fb_{nm}": fb,
    }


def iota_input():
    import ml_dtypes

    return np.asarray(
        np.broadcast_to(np.arange(P, dtype=np.float32), (P, P)).astype(ml_dtypes.bfloat16)
    )


def ref_relation(x, src, dst, n_src, n_dst, W, b, Wl, bl):
    """numpy reference for one relation (float64 for comparison)."""
    deg_s = np.maximum(np.bincount(src, minlength=n_src), 1).astype(np.float64)
    deg_d = np.maximum(np.bincount(dst, minlength=n_dst), 1).astype(np.float64)
    h = x.astype(np.float64) / np.sqrt(deg_s)[:, None]
    agg = np.zeros((n_dst, x.shape[1]))
    np.add.at(agg, dst, h[src])
    agg /= np.sqrt(deg_d)[:, None]
    h1 = agg @ W.astype(np.float64) + b.astype(np.float64)
    h1 = np.where(h1 > 0, h1, 0.01 * h1)
    return h1 @ Wl.astype(np.float64) + bl.astype(np.float64)


from concourse.bass_utils import run_bass_kernel_spmd

N_SVC, N_NODE, N_POD = 50000, 20000, 100000
_PROG_CACHE = {}


def kernel(x_svc, x_pod, x_node,
           svc_src, svc_dst, pod_node_src, pod_node_dst,
           node_pod_src, node_pod_dst,
           W_call, b_call, W_in, b_in, W_ni, b_ni,
           W_lin_svc, b_lin_svc, W_lin_node, b_lin_node,
           W_lin_pod, b_lin_pod):
    x_svc = np.ascontiguousarray(np.asarray(x_svc, np.float32))
    x_pod = np.ascontiguousarray(np.asarray(x_pod, np.float32))
    x_node = np.ascontiguousarray(np.asarray(x_node, np.float32))

    # relation name -> (src table, src_idx, dst_idx, n_src, n_dst, W, b, Wl, bl)
    relspec = [
        ("svc", x_svc, svc_src, svc_dst, N_SVC, N_SVC, W_call, b_call, W_lin_svc, b_lin_svc),
        ("node", x_pod, pod_node_src, pod_node_dst, N_POD, N_NODE, W_in, b_in, W_lin_node, b_lin_node),
        ("pod", x_node, node_pod_src, node_pod_dst, N_NODE, N_POD, W_ni, b_ni, W_lin_pod, b_lin_pod),
    ]

    packs = {}
    rels = []
    for nm, tabx, src, dst, n_src, n_dst, W, b, Wl, bl in relspec:
        pk = pack_relation(np.asarray(src), np.asarray(dst), n_src, n_dst)
        packs[nm] = pk
        rels.append(dict(name=nm, tab_shape=tabx.shape, counts=pk["counts"],
                         totc=pk["totc"], nblk=pk["nblk"]))

    key = tuple((r["name"], tuple(r["counts"])) for r in rels)
    if key not in _PROG_CACHE:
        _PROG_CACHE[key] = build_program(rels)
    nc = _PROG_CACHE[key]

    in_maps = []
    for core in range(N_CORES):
        m = {"iota": iota_input()}
        for nm, tabx, src, dst, n_src, n_dst, W, b, Wl, bl in relspec:
            m.update(make_rel_inputs(nm, packs[nm], tabx,
                                     np.asarray(W, np.float32), np.asarray(b, np.float32),
                                     np.asarray(Wl, np.float32), np.asarray(bl, np.float32), core))
        in_maps.append(m)

    res = run_bass_kernel_spmd(nc, in_maps, core_ids=list(range(N_CORES)), trace=False)

    out = np.empty((N_SVC + N_NODE + N_POD, 64), np.float32)
    row0 = {"svc": 0, "node": N_SVC, "pod": N_SVC + N_NODE}
    for nm, tabx, src, dst, n_src, n_dst, W, b, Wl, bl in relspec:
        pc = packs[nm]["per_core"]
        base = row0[nm]
        for core in range(N_CORES):
            got = res.results[core][f"outT_{nm}"].T[:pc]
            out[base + core * pc : base + (core + 1) * pc] = got
    return out
